# revision 1
# baseline (speedup 1.0000x reference)
"""Causal self-attention, tensor-parallel over heads across 8 NeuronCores.

Reference computation (per problem):
    qkv = x @ w_attn + b_attn ; split q,k,v ; per-head causal softmax attention
    y = att @ v ; out = y @ w_proj + b_proj
Shapes: x [4, 2048, 1024], H=16 heads, head_size=64.

Sharding: 2 heads per core (tensor parallel). Each core computes
    qkv for its heads, causal attention, and a partial y @ w_proj[rows].
Host sums the 8 partial outputs and adds b_proj (the TP all-reduce,
done host-side during the gather step).

Per-core kernel design (all matmuls bf16 with fp32 PSUM accumulate):
  - x is passed pre-transposed (xT [C, B*T]) so every matmul operand is
    already in [contraction, free] layout; no on-device transposes of x.
  - Phase 1: qkvT [384, B*T] = w_qkv.T @ x.T computed in transposed
    layout (q pre-scaled by 1/sqrt(hs) on host, folded into w/b).
  - Phase 1.5: vT -> v (natural layout) via PE transposes, augmented with
    a ones column so the attention-value matmul also produces the
    softmax denominator for free (row 64 of the output).
  - Phase 2: per (batch, head, 512-wide q-group): S^T tiles [128 k-tok,
    512 q] = kT.T @ qT (only causal j-tiles), exp on ScalarE (batched
    over 2 PSUM banks per instruction), multiplicative 0/1 mask on the
    4 diagonal tiles, then y^T[65, 512] += v_aug.T @ P^T.  Softmax skips
    max-subtraction: scores are O(+-8) so exp is safely in fp32 range.
    Normalization: reciprocal of den row, broadcast to 64 partitions via
    a K=1 f32r matmul, multiply on VectorE -> y_proj bf16.
  - Phase 3: partial_out[tok,:] = y_h0.T @ w_proj[h0 rows] + y_h1.T @
    w_proj[h1 rows] accumulated in PSUM, stored bf16.
"""

import math
import numpy as np

try:
    import concourse.bass as bass
except ImportError:  # pragma: no cover
    import sys

    sys.path.insert(0, "/opt/trn_rl_repo")
    import concourse.bass as bass

import ml_dtypes
import concourse.mybir as mybir
import concourse.tile as tile
from concourse import bacc
from concourse.bass_utils import run_bass_kernel_spmd
from concourse.masks import make_identity

BF16 = mybir.dt.bfloat16
F32 = mybir.dt.float32
F32R = mybir.dt.float32r

NCORES = 8


def build_nc(B=4, T=2048, C=1024, H=16, bass_kwargs=None):
    HS = C // H          # 64 head size
    HPC = H // NCORES    # 2 heads per core
    DC = HPC * HS        # 128 local channels
    NT = B * T           # tokens
    CK = C // 128        # contraction chunks for qkv
    QG = 512             # q-group width
    NQG = T // QG        # q-groups per batch
    NJT = T // 128       # 128-wide key tiles per batch
    JT_PER_QG = QG // 128

    assert DC == 128 and HPC == 2

    kw = dict(target_bir_lowering=False, debug=False)
    kw.update(bass_kwargs or {})
    nc = bacc.Bacc("TRN2", **kw)
    xT = nc.dram_tensor("xT", [C, NT], BF16, kind="ExternalInput")
    w_qkv = nc.dram_tensor("w_qkv", [C, 3 * DC], BF16, kind="ExternalInput")
    b_qkv = nc.dram_tensor("b_qkv", [3 * DC], F32, kind="ExternalInput")
    w_proj = nc.dram_tensor("w_proj", [DC, C], BF16, kind="ExternalInput")
    outp = nc.dram_tensor("outp", [NT, C], BF16, kind="ExternalOutput")

    with tile.TileContext(nc) as tc, tc.tile_pool(name="singles", bufs=1) as sg:
        # ---- persistent SBUF ----
        qT_sb = sg.tile([128, NT], BF16)   # rows 0-63 h0, 64-127 h1
        kT_sb = sg.tile([128, NT], BF16)
        vT_sb = sg.tile([128, NT], BF16)
        # v in natural layout, one [128, 65] tile per (b, h, j-tile);
        # col 64 is the ones column (softmax denominator trick)
        v_sb = sg.tile([128, B, HPC, NJT, 65], BF16)
        yp_sb = [sg.tile([64, NT], BF16, name=f"yp{h}_sb") for h in range(HPC)]
        w_sb = sg.tile([128, CK, 3 * DC], BF16)
        wp_sb = [sg.tile([64, C], BF16, name=f"wp{h}_sb") for h in range(HPC)]
        bias_sb = sg.tile([128, 3], F32)
        masks_sb = sg.tile([128, JT_PER_QG, QG], BF16)
        ident_sb = sg.tile([128, 128], BF16)
        ones_f32 = sg.tile([1, 64], F32)
        ones_sb = sg.tile([1, 64], F32R)

        # ---- setup ----
        nc.sync.dma_start(
            out=w_sb, in_=w_qkv.rearrange("(ck p) m -> p ck m", p=128)
        )
        nc.sync.dma_start(
            out=bias_sb, in_=b_qkv.rearrange("(c p) -> p c", p=128)
        )
        for h in range(HPC):
            nc.sync.dma_start(out=wp_sb[h], in_=w_proj[64 * h : 64 * h + 64, :])
        make_identity(nc, ident_sb)
        nc.vector.memset(ones_f32, 1.0)
        with nc.allow_low_precision("f32r ones for denom broadcast"):
            nc.vector.tensor_copy(out=ones_sb, in_=ones_f32)
        # mask[s][j, q] = 1.0 if q >= 128*s + j else 0  (causal, diagonal tiles)
        for s in range(JT_PER_QG):
            nc.gpsimd.memset(masks_sb[:, s, :], 1.0)
            nc.gpsimd.affine_select(
                out=masks_sb[:, s, :],
                in_=masks_sb[:, s, :],
                compare_op=mybir.AluOpType.is_ge,
                fill=0.0,
                base=-128 * s,
                pattern=[[1, QG]],
                channel_multiplier=-1,
            )

        with (
            tc.tile_pool(name="xt_pool", bufs=2) as xt_pool,
            tc.tile_pool(name="pt_pool", bufs=3) as pt_pool,
            tc.tile_pool(name="rd_pool", bufs=2) as rd_pool,
            tc.tile_pool(name="ob_pool", bufs=4) as ob_pool,
            tc.tile_pool(name="ps_stage", bufs=2, space="PSUM") as ps_stage,
            tc.tile_pool(name="ps_yt", bufs=2, space="PSUM") as ps_yt,
            tc.tile_pool(name="ps_misc", bufs=2, space="PSUM") as ps_misc,
        ):
            # ---- phase 1: qkvT = w.T @ xT  (transposed layout) ----
            xT_r = xT.rearrange("(ck p) n -> p ck n", p=128)
            dests = [qT_sb, kT_sb, vT_sb]
            for g in range(NT // 512):
                xt = xt_pool.tile([128, CK, 512], BF16, tag="xt")
                nc.sync.dma_start(out=xt, in_=xT_r[:, :, 512 * g : 512 * g + 512])
                for cc in range(3):
                    qkv_ps = ps_misc.tile([128, 512], F32, tag="misc")
                    for k in range(CK):
                        nc.tensor.matmul(
                            qkv_ps,
                            w_sb[:, k, 128 * cc : 128 * cc + 128],
                            xt[:, k, :],
                            start=(k == 0),
                            stop=(k == CK - 1),
                        )
                    nc.vector.tensor_scalar_add(
                        out=dests[cc][:, 512 * g : 512 * g + 512],
                        in0=qkv_ps,
                        scalar1=bias_sb[:, cc : cc + 1],
                    )

            # ---- phase 1.5: v natural tiles via PE transpose ----
            for b in range(B):
                for h in range(HPC):
                    for jt in range(NJT):
                        vt_ps = ps_misc.tile([128, 64], BF16, tag="misc")
                        nc.tensor.transpose(
                            vt_ps,
                            vT_sb[64 * h : 64 * h + 64,
                                  b * T + 128 * jt : b * T + 128 * jt + 128],
                            ident_sb[64 * h : 64 * h + 64, 64 * h : 64 * h + 64],
                        )
                        nc.vector.tensor_copy(
                            out=v_sb[:, b, h, jt, 0:64], in_=vt_ps
                        )
                        nc.vector.memset(v_sb[:, b, h, jt, 64:65], 1.0)

            # ---- phase 2: causal attention per (b, q-group), heads packed ----
            # The two heads' K=64 S-matmuls sit on disjoint PE row-groups
            # (partitions 0-63 / 64-127) and run concurrently.
            for b in range(B):
                for qg in range(NQG):
                    q0 = b * T + QG * qg
                    njt = JT_PER_QG * (qg + 1)  # causal j-tiles
                    yts = []
                    for h in range(HPC):
                        yt = ps_yt.tile([65, 512], F32, tag="yt",
                                        name=f"yt{h}")
                        yts.append(yt)
                    for jt in range(njt):
                        st = ps_stage.tile([128, 2, 512], F32, tag="stage")
                        pt = pt_pool.tile([128, 2, 512], BF16, tag="pt")
                        for h in range(HPC):
                            hl = slice(64 * h, 64 * h + 64)
                            nc.tensor.matmul(
                                st[:, h, :],
                                kT_sb[hl, b * T + 128 * jt : b * T + 128 * jt + 128],
                                qT_sb[hl, q0 : q0 + QG],
                                start=True,
                                stop=True,
                            )
                        nc.scalar.activation(
                            out=pt, in_=st,
                            func=mybir.ActivationFunctionType.Exp,
                        )
                        diag = jt >= JT_PER_QG * qg
                        for h in range(HPC):
                            if diag:  # diagonal tile: causal 0/1 mask
                                nc.vector.tensor_mul(
                                    pt[:, h, :],
                                    pt[:, h, :],
                                    masks_sb[:, jt - JT_PER_QG * qg, :],
                                )
                            nc.tensor.matmul(
                                yts[h],
                                v_sb[:, b, h, jt, :],
                                pt[:, h, :],
                                start=(jt == 0),
                                stop=(jt == njt - 1),
                            )
                    # normalize: y / den  (den = row 64)
                    for h in range(HPC):
                        rd = rd_pool.tile([1, 512], F32R, tag="rd")
                        with nc.allow_low_precision("f32r denom recip"):
                            nc.vector.reciprocal(rd, yts[h][64:65, :])
                        bc = ps_misc.tile([64, 512], F32, tag="misc")
                        nc.tensor.matmul(
                            bc,
                            ones_sb,
                            rd,
                            start=True,
                            stop=True,
                        )
                        # DVE can read only one PSUM operand: stage bc in SBUF
                        bc_sb = rd_pool.tile([64, 512], F32, tag="bc_sb")
                        nc.vector.tensor_copy(out=bc_sb, in_=bc)
                        nc.vector.tensor_mul(
                            out=yp_sb[h][:, q0 : q0 + QG],
                            in0=yts[h][0:64, :],
                            in1=bc_sb,
                        )

            # ---- phase 3: partial out = y.T @ w_proj (rows) ----
            for tt in range(NT // 128):
                pr = ps_stage.tile([128, 2, 512], F32, tag="stage")
                for ns in range(2):
                    for h in range(HPC):
                        nc.tensor.matmul(
                            pr[:, ns, :],
                            yp_sb[h][:, 128 * tt : 128 * tt + 128],
                            wp_sb[h][:, 512 * ns : 512 * ns + 512],
                            start=(h == 0),
                            stop=(h == HPC - 1),
                        )
                ob = ob_pool.tile([128, 1024], BF16, tag="ob")
                nc.vector.tensor_copy(out=ob, in_=pr)
                nc.sync.dma_start(
                    out=outp[128 * tt : 128 * tt + 128, :], in_=ob
                )

    nc.compile()
    return nc


_NC_CACHE = {}


def _get_nc(shape_key):
    if shape_key not in _NC_CACHE:
        _NC_CACHE[shape_key] = build_nc(*shape_key)
    return _NC_CACHE[shape_key]


def make_in_maps(x, w_attn, b_attn, w_proj, B, T, C, H):
    HS = C // H
    HPC = H // NCORES
    DC = HPC * HS
    scale = 1.0 / math.sqrt(HS)
    bf = ml_dtypes.bfloat16

    xT = np.ascontiguousarray(
        x.reshape(B * T, C).T.astype(bf)
    )
    # w_attn columns: [q | k | v] each [C, C]; head h uses cols h*HS:(h+1)*HS
    wq = w_attn[:, 0:C].reshape(C, H, HS) * scale
    wk = w_attn[:, C : 2 * C].reshape(C, H, HS)
    wv = w_attn[:, 2 * C : 3 * C].reshape(C, H, HS)
    bq = b_attn[0:C].reshape(H, HS) * scale
    bk = b_attn[C : 2 * C].reshape(H, HS)
    bv = b_attn[2 * C :].reshape(H, HS)
    wp = w_proj.reshape(H, HS, C)

    in_maps = []
    for core in range(NCORES):
        hs_ = slice(HPC * core, HPC * core + HPC)
        w_qkv = np.concatenate(
            [
                wq[:, hs_, :].reshape(C, DC),
                wk[:, hs_, :].reshape(C, DC),
                wv[:, hs_, :].reshape(C, DC),
            ],
            axis=1,
        ).astype(bf)
        b_qkv = np.concatenate(
            [
                bq[hs_].reshape(DC),
                bk[hs_].reshape(DC),
                bv[hs_].reshape(DC),
            ]
        ).astype(np.float32)
        wp_core = np.ascontiguousarray(wp[hs_].reshape(DC, C).astype(bf))
        in_maps.append(
            {
                "xT": xT,
                "w_qkv": np.ascontiguousarray(w_qkv),
                "b_qkv": np.ascontiguousarray(b_qkv),
                "w_proj": wp_core,
            }
        )
    return in_maps


def kernel(x, w_attn, b_attn, w_proj, b_proj, _trace=False):
    x = np.asarray(x, dtype=np.float32)
    w_attn = np.asarray(w_attn, dtype=np.float32)
    b_attn = np.asarray(b_attn, dtype=np.float32)
    w_proj = np.asarray(w_proj, dtype=np.float32)
    b_proj = np.asarray(b_proj, dtype=np.float32)

    B, T, C = x.shape
    H = 16
    nc = _get_nc((B, T, C, H))
    in_maps = make_in_maps(x, w_attn, b_attn, w_proj, B, T, C, H)
    res = run_bass_kernel_spmd(
        nc, in_maps, list(range(NCORES)), trace=_trace
    )
    partials = np.stack(
        [res.results[c]["outp"].astype(np.float32) for c in range(NCORES)]
    )
    out = partials.sum(axis=0) + b_proj[None, :]
    if _trace:
        return out.reshape(B, T, C), res
    return out.reshape(B, T, C)



# revision 16
# speedup vs baseline: 40.3602x; 40.3602x over previous
"""Causal self-attention, tensor-parallel over heads across 8 NeuronCores.

Reference computation (per problem):
    qkv = x @ w_attn + b_attn ; split q,k,v ; per-head causal softmax attention
    y = att @ v ; out = y @ w_proj + b_proj
Shapes: x [4, 2048, 1024], H=16 heads, head_size=64.

Sharding: 2 heads per core (tensor parallel). Each core computes
    qkv for its heads, causal attention, and a partial y @ w_proj[rows].
Host sums the 8 partial outputs and adds b_proj (the TP all-reduce,
done host-side during the gather step).

Per-core kernel design (all matmuls bf16 with fp32 PSUM accumulate):
  - x is passed pre-transposed (xT [C, B*T]) so every matmul operand is
    already in [contraction, free] layout; no on-device transposes of x.
  - Phase 1: qkvT [384, B*T] = w_qkv.T @ x.T computed in transposed
    layout (q pre-scaled by 1/sqrt(hs) on host, folded into w/b).
  - Phase 1.5: vT -> v (natural layout) via full 128-row PE transposes
    (both heads per transpose), augmented with a ones column so the
    attention-value matmul also produces the softmax denominator for
    free (row 64 of the output).
  - Phase 2: per (batch, head, 512-wide q-group): S^T tiles [128 k-tok,
    512 q] = kT.T @ qT (only causal j-tiles; the two heads' K=64
    matmuls sit on disjoint PE row-groups and run concurrently), exp on
    ScalarE (batched over 2 PSUM banks per instruction), causal zeroing
    of the 4 diagonal tiles via one GpSimd affine_select over both
    heads, then y^T[65, 512] += v_aug.T @ P^T.  Softmax skips
    max-subtraction: scores are O(+-8) so exp is safely in fp32 range.
    Normalization: reciprocal_approx_fast of the den row, broadcast to
    64 partitions on GpSimd (partition_broadcast), multiply on VectorE
    into the head-stacked y buffer (bf16).
  - Phase 3: partial_out[tok,:] = y_cat.T @ w_proj with both heads
    stacked on the contraction dim (K=128, single matmul per output
    tile, bf16 PSUM), then one VectorE copy + DMA per 128-token tile.
"""

import math
import numpy as np

try:
    import concourse.bass as bass
except ImportError:  # pragma: no cover
    import sys

    sys.path.insert(0, "/opt/trn_rl_repo")
    import concourse.bass as bass

import ml_dtypes
import concourse.mybir as mybir
import concourse.tile as tile
from concourse import bacc
from concourse.bass_utils import run_bass_kernel_spmd
from concourse.masks import make_identity

BF16 = mybir.dt.bfloat16
F32 = mybir.dt.float32
F32R = mybir.dt.float32r

NCORES = 8


def build_nc(B=4, T=2048, C=1024, H=16, bass_kwargs=None):
    HS = C // H          # 64 head size
    HPC = H // NCORES    # 2 heads per core
    DC = HPC * HS        # 128 local channels
    NT = B * T           # tokens
    CK = C // 128        # contraction chunks for qkv
    QG = 512             # q-group width
    NQG = T // QG        # q-groups per batch
    NJT = T // 128       # 128-wide key tiles per batch
    JT_PER_QG = QG // 128

    assert DC == 128 and HPC == 2

    kw = dict(target_bir_lowering=False, debug=False)
    kw.update(bass_kwargs or {})
    nc = bacc.Bacc("TRN2", **kw)
    xT = nc.dram_tensor("xT", [C, NT], BF16, kind="ExternalInput")
    w_qkv = nc.dram_tensor("w_qkv", [C, 3 * DC], BF16, kind="ExternalInput")
    b_qkv = nc.dram_tensor("b_qkv", [3 * DC], F32, kind="ExternalInput")
    w_proj = nc.dram_tensor("w_proj", [DC, C], BF16, kind="ExternalInput")
    outp = nc.dram_tensor("outp", [NT, C], BF16, kind="ExternalOutput")

    with tile.TileContext(nc) as tc, tc.tile_pool(name="singles", bufs=1) as sg:
        # ---- persistent SBUF ----
        qT_sb = sg.tile([128, NT], BF16)   # rows 0-63 h0, 64-127 h1
        kT_sb = sg.tile([128, NT], BF16)
        vT_sb = sg.tile([128, NT], BF16)
        # v in natural layout per (b, j-tile): cols [h0 | ones | h1 | ones]
        # so head h's attention-value stationary is the contiguous 65-col
        # slice [65h : 65h+65]; the ones column doubles as the softmax
        # denominator accumulator (row 64 of the output)
        v_sb = sg.tile([128, B, NJT, 130], BF16)
        # normalized y^T, heads stacked on partitions (phase-3 lhsT)
        yp_sb = sg.tile([128, NT], BF16)
        w_sb = sg.tile([128, CK, 3 * DC], BF16)
        wp_sb = sg.tile([128, C], BF16)
        bias_sb = sg.tile([128, 3], F32)
        ident_sb = sg.tile([128, 128], BF16)
        # causal mask for diagonal tiles (after narrowing to q>=128s the
        # predicate is the same for every s): mask[ch, j] = 1 if j >= ch
        mask_sb = sg.tile([128, QG], BF16)
        ones_f32 = sg.tile([1, 64], F32)
        ones_sb = sg.tile([1, 64], F32R)

        # ---- setup ----
        nc.sync.dma_start(
            out=w_sb, in_=w_qkv.rearrange("(ck p) m -> p ck m", p=128)
        )
        nc.sync.dma_start(
            out=bias_sb, in_=b_qkv.rearrange("(c p) -> p c", p=128)
        )
        nc.sync.dma_start(out=wp_sb, in_=w_proj[:, :])
        make_identity(nc, ident_sb)
        # ones columns of every v tile (cols 64 and 129), one strided memset
        nc.gpsimd.memset(v_sb.rearrange("p b j (g c) -> p b j g c", c=65)[:, :, :, :, 64:65], 1.0)
        nc.gpsimd.memset(mask_sb, 1.0)
        nc.gpsimd.affine_select(
            out=mask_sb,
            in_=mask_sb,
            compare_op=mybir.AluOpType.is_ge,
            fill=0.0,
            base=0,
            pattern=[[1, QG]],
            channel_multiplier=-1,
        )
        nc.vector.memset(ones_f32, 1.0)
        with nc.allow_low_precision("f32r ones for denom broadcast"):
            nc.vector.tensor_copy(out=ones_sb, in_=ones_f32)

        with (
            tc.tile_pool(name="xt_pool", bufs=2) as xt_pool,
            tc.tile_pool(name="pt_pool", bufs=3) as pt_pool,
            tc.tile_pool(name="rd_pool", bufs=2) as rd_pool,
            tc.tile_pool(name="bc_pool", bufs=2) as bc_pool,
            tc.tile_pool(name="ob_pool", bufs=4) as ob_pool,
        ):
            # ---- fused pipeline: per 512-token group g = 4*b+qg ----
            #   stage A: qkv for group g (PE-heavy, feeds SBUF)
            #   stage B: v transposes for group g
            #   stage C: causal attention + c_proj partials for group g-1
            # Emitting A/B for g before C for g-1 lets the scheduler overlap
            # the PE-bound qkv matmuls with the ACT-bound attention exps.
            xT_r = xT.rearrange("(ck p) n -> p ck n", p=128)
            dests = [qT_sb, kT_sb, vT_sb]
            v_sb_r = v_sb.rearrange("p b j (g c) -> p b j g c", c=65)
            outp_r = outp.rearrange("(u j p) c -> u p j c", p=128, j=2)
            NG = NT // 512
            with (
                tc.tile_pool(name="ps_in", bufs=2, space="PSUM") as ps_in,
                tc.tile_pool(name="ps_stage", bufs=2, space="PSUM") as ps_stage,
                tc.tile_pool(name="ps_yt", bufs=2, space="PSUM") as ps_yt,
            ):
                for g in range(NG + 1):
                    if g < NG:
                        # ---- stage A: qkv for group g ----
                        xt = xt_pool.tile([128, CK, 512], BF16, tag="xt")
                        nc.sync.dma_start(
                            out=xt, in_=xT_r[:, :, 512 * g : 512 * g + 512]
                        )
                        for cc in range(3):
                            qkv_ps = ps_in.tile([128, 512], F32, tag="psin")
                            for k in range(CK):
                                nc.tensor.matmul(
                                    qkv_ps,
                                    w_sb[:, k, 128 * cc : 128 * cc + 128],
                                    xt[:, k, :],
                                    start=(k == 0),
                                    stop=(k == CK - 1),
                                )
                            nc.vector.tensor_scalar_add(
                                out=dests[cc][:, 512 * g : 512 * g + 512],
                                in0=qkv_ps,
                                scalar1=bias_sb[:, cc : cc + 1],
                            )
                        # ---- stage B: v transposes for group g ----
                        b, qg = divmod(g, NQG)
                        for sj in range(JT_PER_QG):
                            jt = JT_PER_QG * qg + sj
                            vt_ps = ps_in.tile([128, 2, 64], BF16, tag="psin")
                            nc.tensor.transpose(
                                vt_ps,
                                vT_sb[:, b * T + 128 * jt : b * T + 128 * jt + 128],
                                ident_sb,
                            )
                            # both heads in one copy: dest cols {0:64, 65:129}
                            nc.vector.tensor_copy(
                                out=v_sb_r[:, b, jt, :, 0:64],
                                in_=vt_ps,
                            )
                    if g == 0:
                        continue
                    # ---- stage C: attention + c_proj for group g-1 ----
                    b, qg = divmod(g - 1, NQG)
                    q0 = b * T + QG * qg
                    njt = JT_PER_QG * (qg + 1)  # causal j-tiles
                    yts = []
                    for h in range(HPC):
                        yt = ps_yt.tile([65, 512], F32, tag="yt",
                                        name=f"yt{h}")
                        yts.append(yt)
                    for jt in range(njt):
                        st = ps_stage.tile([128, 2, 512], F32, tag="stage")
                        pt = pt_pool.tile([128, 2, 512], BF16, tag="pt")
                        # diagonal tiles only need q >= 128*s (causal);
                        # narrow every stage to the live q-range
                        s = jt - JT_PER_QG * qg
                        lo = 128 * s if s > 0 else 0
                        w = QG - lo
                        for h in range(HPC):
                            hl = slice(64 * h, 64 * h + 64)
                            nc.tensor.matmul(
                                st[:, h, lo:QG],
                                kT_sb[hl,
                                      b * T + 128 * jt : b * T + 128 * jt + 128],
                                qT_sb[hl, q0 + lo : q0 + QG],
                                start=True,
                                stop=True,
                            )
                        nc.scalar.activation(
                            out=pt[:, :, lo:QG], in_=st[:, :, lo:QG],
                            func=mybir.ActivationFunctionType.Exp,
                        )
                        if s >= 0:  # diagonal tile: causal zeroing
                            nc.vector.tensor_mul(
                                pt[:, :, lo:QG],
                                pt[:, :, lo:QG],
                                mask_sb[:, 0:w].unsqueeze(1)
                                .broadcast_to([128, 2, w]),
                            )
                        for h in range(HPC):
                            nc.tensor.matmul(
                                yts[h][:, lo:QG],
                                v_sb_r[:, b, jt, h, :],
                                pt[:, h, lo:QG],
                                start=(jt == 0),
                                stop=(jt == njt - 1),
                                skip_group_check=True,
                            )
                    # normalize: y / den  (den = row 64)
                    for h in range(HPC):
                        rd = rd_pool.tile([1, 512], F32R, tag="rd")
                        with nc.allow_low_precision("f32r denom recip"):
                            nc.vector.reciprocal(rd, yts[h][64:65, :])
                        bc_ps = ps_in.tile([64, 512], F32, tag="psin")
                        nc.tensor.matmul(
                            bc_ps, ones_sb, rd, start=True, stop=True,
                        )
                        # DVE can read only one PSUM operand: stage in SBUF
                        bc = bc_pool.tile([64, 512], F32, tag="bc")
                        nc.vector.tensor_copy(out=bc, in_=bc_ps)
                        nc.vector.tensor_mul(
                            out=yp_sb[64 * h : 64 * h + 64, q0 : q0 + QG],
                            in0=yts[h][0:64, :],
                            in1=bc,
                        )
                    # c_proj partials for this q-group's 4 token tiles
                    for uu in range(2):
                        u = (b * NQG + qg) * 2 + uu
                        ob = ob_pool.tile([128, 2, 1024], BF16, tag="ob")
                        for j in range(2):
                            for ns in range(2):
                                pr = ps_in.tile([128, 512], F32, tag="psin")
                                nc.tensor.matmul(
                                    pr,
                                    yp_sb[:, 256 * u + 128 * j :
                                          256 * u + 128 * j + 128],
                                    wp_sb[:, 512 * ns : 512 * ns + 512],
                                    start=True,
                                    stop=True,
                                )
                                # downconvert mostly on DVE (ACT is
                                # exp-bound here), 1 in 4 on ACT
                                if (j, ns) == (1, 1):
                                    nc.scalar.activation(
                                        out=ob[:, j, 512 * ns : 512 * ns + 512],
                                        in_=pr,
                                        func=mybir.ActivationFunctionType.Copy,
                                    )
                                else:
                                    nc.vector.tensor_copy(
                                        out=ob[:, j, 512 * ns : 512 * ns + 512],
                                        in_=pr,
                                    )
                        nc.sync.dma_start(out=outp_r[u], in_=ob)

    nc.compile()
    return nc


_NC_CACHE = {}


def _get_nc(shape_key):
    if shape_key not in _NC_CACHE:
        _NC_CACHE[shape_key] = build_nc(*shape_key)
    return _NC_CACHE[shape_key]


def make_in_maps(x, w_attn, b_attn, w_proj, B, T, C, H):
    HS = C // H
    HPC = H // NCORES
    DC = HPC * HS
    scale = 1.0 / math.sqrt(HS)
    bf = ml_dtypes.bfloat16

    xT = np.ascontiguousarray(
        x.reshape(B * T, C).T.astype(bf)
    )
    # w_attn columns: [q | k | v] each [C, C]; head h uses cols h*HS:(h+1)*HS
    wq = w_attn[:, 0:C].reshape(C, H, HS) * scale
    wk = w_attn[:, C : 2 * C].reshape(C, H, HS)
    wv = w_attn[:, 2 * C : 3 * C].reshape(C, H, HS)
    bq = b_attn[0:C].reshape(H, HS) * scale
    bk = b_attn[C : 2 * C].reshape(H, HS)
    bv = b_attn[2 * C :].reshape(H, HS)
    wp = w_proj.reshape(H, HS, C)

    in_maps = []
    for core in range(NCORES):
        hs_ = slice(HPC * core, HPC * core + HPC)
        w_qkv = np.concatenate(
            [
                wq[:, hs_, :].reshape(C, DC),
                wk[:, hs_, :].reshape(C, DC),
                wv[:, hs_, :].reshape(C, DC),
            ],
            axis=1,
        ).astype(bf)
        b_qkv = np.concatenate(
            [
                bq[hs_].reshape(DC),
                bk[hs_].reshape(DC),
                bv[hs_].reshape(DC),
            ]
        ).astype(np.float32)
        wp_core = np.ascontiguousarray(wp[hs_].reshape(DC, C).astype(bf))
        in_maps.append(
            {
                "xT": xT,
                "w_qkv": np.ascontiguousarray(w_qkv),
                "b_qkv": np.ascontiguousarray(b_qkv),
                "w_proj": wp_core,
            }
        )
    return in_maps


def kernel(x, w_attn, b_attn, w_proj, b_proj, _trace=False):
    x = np.asarray(x, dtype=np.float32)
    w_attn = np.asarray(w_attn, dtype=np.float32)
    b_attn = np.asarray(b_attn, dtype=np.float32)
    w_proj = np.asarray(w_proj, dtype=np.float32)
    b_proj = np.asarray(b_proj, dtype=np.float32)

    B, T, C = x.shape
    H = 16
    nc = _get_nc((B, T, C, H))
    in_maps = make_in_maps(x, w_attn, b_attn, w_proj, B, T, C, H)
    res = run_bass_kernel_spmd(
        nc, in_maps, list(range(NCORES)), trace=_trace
    )
    partials = np.stack(
        [res.results[c]["outp"].astype(np.float32) for c in range(NCORES)]
    )
    out = partials.sum(axis=0) + b_proj[None, :]
    if _trace:
        return out.reshape(B, T, C), res
    return out.reshape(B, T, C)
